# revision 1
# baseline (speedup 1.0000x reference)
"""Trainium2 Bass kernel for the BINN convnet problem.

Computation (per row b of inp, all column indices mod D=128):
    x[b, j]  = (c1[j] * a[b, j+1] - c2[j] * a[b, j-2]) * a[b, j-1]
    out      = x + a @ W_lin.T + b_lin
with c1[j] = w[j,0]*w[j,2], c2[j] = w[j,1]*w[j,2], except j==1 where the
outer factor is w[1,0] instead of w[1,2].

Strategy: pure data parallel across 8 NeuronCores (batch split).  On each
core, per 512-row compute subtile (1024-row DMA tiles, (p q) layout so each
partition line is one contiguous 4 KB DRAM chunk):

  1. PE-transposes A -> A^T per 128-row block (plain fp32 transpose mode);
     ScalarE evacuates PSUM->SBUF, rounding to float32r.
  2. The stencil's linear part g[b,j] = c1[j] a[b,j+1] - c2[j] a[b,j-2] is
     a constant banded matrix G.  One float32r matmul per block with
     lhsT = A^T-block (stationary) and rhs = [G^T | W_lin^T] (moving,
     N=256 -> full PE rate) produces g and mm = a @ W_lin.T both in
     NATURAL layout in PSUM.  No transpose-back is needed.
  3. DVE: x = a[:, j-1] * g with the j-1 roll expressed as shifted
     free-dim access patterns on the natural A tile (main + 1-col wrap),
     then out = x + mm written straight to SBUF.
  4. GpSimd adds the column bias b_lin (broadcast constant) in SBUF.
  5. Store the natural tile.
"""

import os
import sys

import numpy as np

if os.path.isdir("/opt/trn_rl_repo") and "/opt/trn_rl_repo" not in sys.path:
    sys.path.insert(0, "/opt/trn_rl_repo")

import concourse.mybir as mybir
import concourse.tile as tile
from concourse import bacc
from concourse.bass_utils import run_bass_kernel_spmd

D = 128          # feature dim
N_CORES = 8
SUB = 512        # rows per compute subtile
DMA_ROWS = 2048  # rows per DMA tile
F32 = mybir.dt.float32
F32R = mybir.dt.float32r
BIAS_ON_POOL = True


def build_program(nrows: int):
    """Build the single-core Bass program (SPMD across cores)."""
    assert nrows % DMA_ROWS == 0
    ndma = nrows // DMA_ROWS
    nsub = DMA_ROWS // SUB  # compute subtiles per DMA tile (2)
    QB = SUB // D           # 128-row blocks per compute subtile (4)

    nc = bacc.Bacc("TRN2", debug=False, target_bir_lowering=False)

    inp = nc.declare_dram_parameter("inp", [nrows, D], F32, isOutput=False)
    gw = nc.declare_dram_parameter("gw", [D, 2 * D], F32, isOutput=False)
    bbc = nc.declare_dram_parameter("bbc", [D, DMA_ROWS], F32, isOutput=False)
    bmask = nc.declare_dram_parameter("bmask", [1, SUB], F32, isOutput=False)
    ones = nc.declare_dram_parameter("ones", [1, D], F32, isOutput=False)
    ident = nc.declare_dram_parameter("ident", [D, D], F32, isOutput=False)
    out = nc.declare_dram_parameter("out", [nrows, D], F32, isOutput=True)

    with tile.TileContext(nc) as tc:
        with (
            tc.tile_pool(name="const", bufs=1) as const_pool,
            tc.tile_pool(name="a_sb", bufs=6) as a_pool,
            tc.tile_pool(name="at_sb", bufs=4) as at_pool,
            tc.tile_pool(name="xt_sb", bufs=4) as xt_pool,
            tc.tile_pool(name="o_sb", bufs=6) as o_pool,
            tc.tile_pool(name="at_ps", bufs=2, space="PSUM") as atps_pool,
            tc.tile_pool(name="gw_ps", bufs=3, space="PSUM") as gwps_pool,
        ):
            # --- constants, loaded once ---
            gw_sb = const_pool.tile([D, 2 * D], F32)
            bbc_sb = const_pool.tile([D, DMA_ROWS], F32)
            bmask_sb = const_pool.tile([1, SUB], F32)
            ones_sb = const_pool.tile([1, D], F32)
            id_sb = const_pool.tile([D, D], F32)
            nc.sync.dma_start(out=gw_sb[:], in_=gw[:, :])
            nc.sync.dma_start(out=bbc_sb[:], in_=bbc[:, :])
            nc.sync.dma_start(out=bmask_sb[:], in_=bmask[:, :])
            nc.sync.dma_start(out=ones_sb[:], in_=ones[:, :])
            nc.sync.dma_start(out=id_sb[:], in_=ident[:, :])

            # fp32r matmul operands must be produced by an fp32r-rounding
            # instruction (walrus checkMatmultFP32r) — round the constants once.
            gw_rt = const_pool.tile([D, 2 * D], F32R)
            bmask_rt = const_pool.tile([1, SUB], F32R)
            ones_rt = const_pool.tile([1, D], F32R)
            id_rt = const_pool.tile([D, D], F32R)
            nc.vector.tensor_copy(out=gw_rt[:], in_=gw_sb[:])
            nc.vector.tensor_copy(out=bmask_rt[:], in_=bmask_sb[:])
            nc.vector.tensor_copy(out=ones_rt[:], in_=ones_sb[:])
            nc.vector.tensor_copy(out=id_rt[:], in_=id_sb[:])

            # Software pipeline by one subtile: PE's stream per step is
            # [trA(k), GW(k-1)] so PE transposes subtile k while ScalarE
            # evacuates A^T of k-1 — no PE stall on the evac round-trip.
            nsubs = ndma * nsub
            st = {}  # k -> (td, f0, a_sb, o_sb, at_ps, at_sb)

            def emit_front(k):
                td, ts = divmod(k, nsub)
                if ts == 0:
                    r0 = td * DMA_ROWS
                    # (p q) layout: partition p holds DMA_ROWS/128 consecutive
                    # DRAM rows -> one contiguous DRAM chunk per partition.
                    a_sb = a_pool.tile([D, DMA_ROWS], F32, tag="a")
                    src = inp[r0 : r0 + DMA_ROWS, :].rearrange(
                        "(p q) d -> p q d", p=D
                    )
                    nc.sync.dma_start(
                        out=a_sb[:].rearrange("p (q d) -> p q d", d=D), in_=src
                    )
                    o_sb = o_pool.tile([D, DMA_ROWS], F32, tag="o")
                else:
                    _, _, a_sb, o_sb, _, _ = st[k - 1]
                f0 = ts * SUB

                # --- PE transpose A -> A^T (per 128 block, plain fp32) ---
                at_ps = atps_pool.tile([D, SUB], F32, tag="atps")
                for q in range(QB):
                    nc.tensor.matmul(
                        out=at_ps[:, q * D : (q + 1) * D],
                        lhsT=a_sb[:, f0 + q * D : f0 + (q + 1) * D],
                        rhs=id_sb[:],
                        is_transpose=True,
                        start=True,
                        stop=True,
                    )
                st[k] = (td, f0, a_sb, o_sb, at_ps, None)

            def emit_evac(k):
                td, f0, a_sb, o_sb, at_ps, _ = st[k]
                # evacuate A^T to SBUF (ScalarE), rounding to fp32r
                at_sb = at_pool.tile([D, SUB], F32R, tag="at")
                nc.scalar.copy(out=at_sb[:], in_=at_ps[:])
                st[k] = (td, f0, a_sb, o_sb, at_sb, None)

            def emit_gw(k):
                td, f0, a_sb, o_sb, at_sb, _ = st[k]
                # --- [g | mm] per block, natural layout, in PSUM ---
                # gw_ps free layout: [g0|m0|g1|m1|g2|m2|g3|m3], 2 banks
                gw_ps = gwps_pool.tile([D, 4 * 2 * D], F32, tag="gwps")
                for q in range(QB):
                    nc.tensor.matmul(
                        out=gw_ps[:, q * 2 * D : (q + 1) * 2 * D],
                        lhsT=at_sb[:, q * D : (q + 1) * D],
                        rhs=gw_rt[:],
                        start=True,
                        stop=BIAS_ON_POOL,
                    )
                if not BIAS_ON_POOL:
                    # accumulate b_lin onto the mm halves (masked rhs)
                    for h in range(2):
                        nc.tensor.matmul(
                            out=gw_ps[:, h * SUB : (h + 1) * SUB],
                            lhsT=ones_rt[:],
                            rhs=bmask_rt[:],
                            start=False,
                            stop=True,
                        )
                st[k] = (td, f0, a_sb, o_sb, at_sb, gw_ps)

            def emit_back(k):
                td, f0, a_sb, o_sb, _, gw_ps = st[k]
                gw3 = gw_ps[:].rearrange("p (q c) -> p q c", c=2 * D)
                a3 = a_sb[:, f0 : f0 + SUB].rearrange("p (q d) -> p q d", d=D)
                o3 = o_sb[:, f0 : f0 + SUB].rearrange("p (q d) -> p q d", d=D)

                # --- x = a[:, j-1] * g (DVE; shifted free-dim APs) ---
                xt_sb = xt_pool.tile([D, SUB], F32, tag="xt")
                x3 = xt_sb[:].rearrange("p (q d) -> p q d", d=D)
                nc.vector.tensor_mul(
                    out=x3[:, :, 1:D], in0=a3[:, :, 0 : D - 1], in1=gw3[:, :, 1:D]
                )
                nc.vector.tensor_mul(
                    out=x3[:, :, 0:1], in0=a3[:, :, D - 1 : D], in1=gw3[:, :, 0:1]
                )

                # --- out = x + mm (DVE, straight to SBUF) ---
                nc.vector.tensor_add(
                    out=o3[:, :, :], in0=xt_sb[:], in1=gw3[:, :, D : 2 * D]
                )

                if BIAS_ON_POOL:
                    # --- += b_lin broadcast (GpSimd, SBUF only) ---
                    nc.gpsimd.tensor_tensor(
                        out=o_sb[:, f0 : f0 + SUB],
                        in0=o_sb[:, f0 : f0 + SUB],
                        in1=bbc_sb[:, 0:SUB],
                        op=mybir.AluOpType.add,
                    )

            def emit_store(k):
                td, _, _, o_sb, _, _ = st[k]
                if k % nsub == nsub - 1:
                    # --- store (Scalar HWDGE ring; loads use the SP ring).
                    # Deferred one extra stage so the store's semaphore wait
                    # (on the GpSimd bias) never stalls ACT's queue ahead of
                    # the next evacuations. ---
                    r0 = td * DMA_ROWS
                    dst = out[r0 : r0 + DMA_ROWS, :].rearrange(
                        "(p q) d -> p q d", p=D
                    )
                    nc.scalar.dma_start(
                        out=dst, in_=o_sb[:].rearrange("p (q d) -> p q d", d=D)
                    )

            # 5-stage pipeline:
            # [trA(k)] [evac(k-1)] [GW(k-2)] [TT+bias(k-3)] [store(k-4)]
            for step in range(nsubs + 4):
                if step < nsubs:
                    emit_front(step)
                if step >= 1 and step - 1 < nsubs:
                    emit_evac(step - 1)
                if step >= 2 and step - 2 < nsubs:
                    emit_gw(step - 2)
                if step >= 3 and step - 3 < nsubs:
                    emit_back(step - 3)
                if step >= 4 and step - 4 < nsubs:
                    emit_store(step - 4)

    nc.compile()
    return nc


def make_consts(w: np.ndarray, W_lin: np.ndarray, b_lin: np.ndarray):
    """Host-side constant preparation (all tiny)."""
    w = np.asarray(w, np.float64)
    c1 = w[:, 0] * w[:, 2]
    c2 = w[:, 1] * w[:, 2]
    # column 1 uses w[1,0] as the outer factor (faithful to source)
    c1[1] = w[1, 0] * w[1, 0]
    c2[1] = w[1, 1] * w[1, 0]

    j = np.arange(D)
    G = np.zeros((D, D), np.float64)
    G[j, (j + 1) % D] += c1
    G[j, (j - 2) % D] -= c2

    gwm = np.zeros((D, 2 * D), np.float32)
    gwm[:, :D] = G.T           # gw[d, j] = G[j, d]
    gwm[:, D:] = np.asarray(W_lin, np.float64).T  # gw[d, D+j] = W_lin[j, d]

    b32 = np.asarray(b_lin, np.float32)
    bbc = np.ascontiguousarray(np.tile(b32, (D, DMA_ROWS // D)))  # [128, 1024]
    bmask = np.zeros((1, SUB), np.float32)
    bmask[0, D : 2 * D] = b32
    bmask[0, 3 * D : 4 * D] = b32
    ones = np.ones((1, D), np.float32)
    ident = np.eye(D, dtype=np.float32)
    return {"gw": gwm, "bbc": bbc, "bmask": bmask, "ones": ones, "ident": ident}


_PROGRAM_CACHE: dict[int, object] = {}
TRACE = False      # test-only: capture NTFF profile on the next kernel() call
TRACE_DIR = None   # test-only: where to keep NTFF/perfetto artifacts
LAST_RESULT = None  # test-only: BassKernelResults of the last run


def _get_program(nrows: int):
    if nrows not in _PROGRAM_CACHE:
        _PROGRAM_CACHE[nrows] = build_program(nrows)
    return _PROGRAM_CACHE[nrows]


def kernel(**inputs) -> np.ndarray:
    inp = np.ascontiguousarray(np.asarray(inputs["inp"], np.float32))
    w = np.asarray(inputs["w"], np.float32)
    W_lin = np.asarray(inputs["W_lin"], np.float32)
    b_lin = np.asarray(inputs["b_lin"], np.float32)

    B = inp.shape[0]
    assert inp.shape[1] == D and B % N_CORES == 0
    nrows = B // N_CORES

    consts = make_consts(w, W_lin, b_lin)
    shards = inp.reshape(N_CORES, nrows, D)

    nc = _get_program(nrows)
    in_maps = [{"inp": shards[i], **consts} for i in range(N_CORES)]
    res = run_bass_kernel_spmd(
        nc, in_maps, list(range(N_CORES)), trace=TRACE, tmpdir=TRACE_DIR
    )
    global LAST_RESULT
    LAST_RESULT = res
    return np.concatenate([res.results[i]["out"] for i in range(N_CORES)], axis=0)


if __name__ == "__main__":
    # quick smoke test on random data vs numpy
    rng = np.random.default_rng(0)
    B = N_CORES * DMA_ROWS * 2
    inp = rng.standard_normal((B, D)).astype(np.float32)
    w = rng.random((D, 3)).astype(np.float32)
    W_lin = (rng.standard_normal((D, D)) / np.sqrt(D)).astype(np.float32)
    b_lin = (rng.standard_normal(D) * 0.01).astype(np.float32)
    dt = np.ones(1, np.float32)

    actual = kernel(inp=inp, dt=dt, w=w, W_lin=W_lin, b_lin=b_lin)

    a = inp.astype(np.float64)
    c1 = (w[:, 0] * w[:, 2]).astype(np.float64)
    c2 = (w[:, 1] * w[:, 2]).astype(np.float64)
    c1[1] = w[1, 0] * w[1, 0]
    c2[1] = w[1, 1] * w[1, 0]
    ap1 = np.roll(a, -1, 1)
    am2 = np.roll(a, 2, 1)
    am1 = np.roll(a, 1, 1)
    x = (c1 * ap1 - c2 * am2) * am1
    expected = x + a @ W_lin.astype(np.float64).T + b_lin
    err = np.abs(actual - expected).max() / np.abs(expected).max()
    print("scale-relative absmax err:", err)



# revision 10
# speedup vs baseline: 1.1211x; 1.1211x over previous
"""Trainium2 Bass kernel for the BINN convnet problem (fp16, transposed layout).

Computation (per row b of inp, all column indices mod D=128):
    g[b, j]  = c1[j] * a[b, j+1] - c2[j] * a[b, j-2]
    x[b, j]  = g[b, j] * a[b, j-1]
    out      = x + a @ W_lin.T + b_lin
with c1[j] = w[j,0]*w[j,2], c2[j] = w[j,1]*w[j,2], except j==1 where the
outer factor is w[1,0] instead of w[1,2].  g is linear in a: g = a @ G.T for
a constant banded G.

The correctness gate is scale-relative absmax < 2e-2; fp16 end-to-end
measures ~7e-4, so all HBM traffic runs at 2 bytes/elem — half the fp32
baseline (32 MiB/core instead of 64 MiB/core; DMA is the roofline).

Strategy: pure data parallel across 8 NeuronCores (batch split).  The host
pre-transposes each 65536-row shard to A^T [128, 65536] fp16, so on-device:

  1. Loads are plain contiguous DMAs (16 KiB per partition per tile),
     no transposes anywhere on device.
  2. g^T = G @ A^T and mm^T = W_lin @ A^T are matmuls with *constant*
     stationary operands (G^T, W_lin^T) and A^T chunks moving, N=512 per
     PSUM bank, natural transposed output layout.
  3. The stencil roll j-1 is absorbed into the constants: the device
     computes the row-rotated output out_dev[p] = out[(p+1) mod 128] using
     rolled G, W_lin, b_lin, so the DVE multiply x_dev = g_rot * A^T is
     perfectly partition-aligned (PSUM partition offsets are illegal) and
     needs no wrap op.  The host un-rotates for free on assembly.
  4. The W matmul writes its own PSUM bank; ScalarE evacuates it to SBUF
     fp16 adding b_lin, which in transposed layout is a per-partition
     activation bias.  (Accumulating W on top of the DVE result via
     start=False raced: the tracker misses that write, so the ScalarE read
     overlapped the accumulation during pipeline warmup.)
  5. GpSimd sums the two SBUF halves: out = x + (mm + b).
  6. fp16 stores; the host transposes back and upcasts to fp32.  Stores are
     emitted on the ACT queue two 2-chunk groups late so their semaphore
     wait (on GpSimd) never stalls ACT's queue ahead of pending evacs.

PE sees [G,G,W,W] per 2-chunk group - one stationary reload per operand
per group; all four compute engines run disjoint stages concurrently.
"""

import os
import sys

import numpy as np

if os.path.isdir("/opt/trn_rl_repo") and "/opt/trn_rl_repo" not in sys.path:
    sys.path.insert(0, "/opt/trn_rl_repo")

import concourse.mybir as mybir
import concourse.tile as tile
from concourse import bacc
from concourse.bass_utils import run_bass_kernel_spmd

D = 128          # feature dim
N_CORES = 8
CHUNK = 512      # columns (= batch rows) per PSUM bank / matmul
TCOLS = 8192     # columns per DMA tile (2 MiB fp16)
F16 = mybir.dt.float16
F32 = mybir.dt.float32


def build_program(ncols: int):
    """Build the single-core Bass program (SPMD across cores).

    ncols = rows of the original problem handled by this core; the device
    works on A^T [128, ncols] fp16.
    """
    assert ncols % TCOLS == 0
    ntiles = ncols // TCOLS
    cpt = TCOLS // CHUNK          # chunks per tile (16)
    nchunks = ntiles * cpt
    GRP = 2                       # chunks per PE stationary group

    nc = bacc.Bacc("TRN2", debug=False, target_bir_lowering=False)

    at_d = nc.declare_dram_parameter("at", [D, ncols], F16, isOutput=False)
    gt_d = nc.declare_dram_parameter("gt", [D, D], F16, isOutput=False)
    wt_d = nc.declare_dram_parameter("wt", [D, D], F16, isOutput=False)
    b_d = nc.declare_dram_parameter("b", [D, 1], F32, isOutput=False)
    out_d = nc.declare_dram_parameter("out", [D, ncols], F16, isOutput=True)

    with tile.TileContext(nc) as tc:
        with (
            tc.tile_pool(name="const", bufs=1) as const_pool,
            tc.tile_pool(name="a_sb", bufs=3) as a_pool,
            tc.tile_pool(name="o_sb", bufs=3) as o_pool,
            tc.tile_pool(name="xt_sb", bufs=4) as xt_pool,
            tc.tile_pool(name="mb_sb", bufs=4) as mb_pool,
            tc.tile_pool(name="g_ps", bufs=4, space="PSUM") as g_pool,
            tc.tile_pool(name="m_ps", bufs=4, space="PSUM") as m_pool,
        ):
            gt_sb = const_pool.tile([D, D], F16)
            wt_sb = const_pool.tile([D, D], F16)
            b_sb = const_pool.tile([D, 1], F32)
            nc.sync.dma_start(out=gt_sb[:], in_=gt_d[:, :])
            nc.sync.dma_start(out=wt_sb[:], in_=wt_d[:, :])
            nc.sync.dma_start(out=b_sb[:], in_=b_d[:, :])

            tiles = {}  # tile t -> (at_sb, o_sb)
            pending_stores = []  # [(emit_after_group, t, o_sb), ...]

            def tile_of(k):
                t, c = divmod(k, cpt)
                if c == 0:
                    at_sb = a_pool.tile([D, TCOLS], F16, tag="at")
                    nc.sync.dma_start(
                        out=at_sb[:], in_=at_d[:, t * TCOLS : (t + 1) * TCOLS]
                    )
                    o_sb = o_pool.tile([D, TCOLS], F16, tag="o")
                    tiles[t] = (at_sb, o_sb)
                return tiles[t]

            ngroups = nchunks // GRP
            for grp in range(ngroups):
                ks = [grp * GRP + i for i in range(GRP)]
                ats, gs, ms, xts, mbs = [], [], [], [], []
                for k in ks:
                    at_sb, o_sb = tile_of(k)
                    col = (k % cpt) * CHUNK
                    ats.append((at_sb, o_sb, col))
                    g_ps = g_pool.tile([D, CHUNK], F32, tag="g")
                    nc.tensor.matmul(
                        out=g_ps[:],
                        lhsT=gt_sb[:],
                        rhs=at_sb[:, col : col + CHUNK],
                        start=True,
                        stop=True,
                    )
                    gs.append(g_ps)
                for (at_sb, o_sb, col), k in zip(ats, ks):
                    m_ps = m_pool.tile([D, CHUNK], F32, tag="m")
                    nc.tensor.matmul(
                        out=m_ps[:],
                        lhsT=wt_sb[:],
                        rhs=at_sb[:, col : col + CHUNK],
                        start=True,
                        stop=True,
                    )
                    ms.append(m_ps)
                # x_dev[p] = g[p+1]*a[p]: rotation baked into G_rot, so this
                # is a single partition-aligned multiply.
                for (at_sb, o_sb, col), g_ps in zip(ats, gs):
                    xt_sb = xt_pool.tile([D, CHUNK], F16, tag="xt")
                    nc.vector.tensor_mul(
                        out=xt_sb[:], in0=g_ps[:], in1=at_sb[:, col : col + CHUNK]
                    )
                    xts.append(xt_sb)
                # mm + b_lin (per-partition bias), PSUM -> SBUF fp16
                for m_ps in ms:
                    mb_sb = mb_pool.tile([D, CHUNK], F16, tag="mb")
                    nc.scalar.add(out=mb_sb[:], in_=m_ps[:], add=b_sb[:, 0:1])
                    mbs.append(mb_sb)
                # deferred stores go on the ACT queue after this group's evacs
                while pending_stores and pending_stores[0][0] <= grp:
                    _, t, o_sb = pending_stores.pop(0)
                    nc.scalar.dma_start(
                        out=out_d[:, t * TCOLS : (t + 1) * TCOLS], in_=o_sb[:]
                    )
                for (at_sb, o_sb, col), xt_sb, mb_sb in zip(ats, xts, mbs):
                    nc.gpsimd.tensor_tensor(
                        out=o_sb[:, col : col + CHUNK],
                        in0=xt_sb[:],
                        in1=mb_sb[:],
                        op=mybir.AluOpType.add,
                    )
                for k in ks:
                    t, c = divmod(k, cpt)
                    if c == cpt - 1:
                        pending_stores.append((grp + 2, t, tiles[t][1]))
            for _, t, o_sb in pending_stores:
                nc.scalar.dma_start(
                    out=out_d[:, t * TCOLS : (t + 1) * TCOLS], in_=o_sb[:]
                )

    nc.compile()
    return nc


def make_consts(w: np.ndarray, W_lin: np.ndarray, b_lin: np.ndarray):
    """Host-side constant preparation (all tiny)."""
    w = np.asarray(w, np.float64)
    c1 = w[:, 0] * w[:, 2]
    c2 = w[:, 1] * w[:, 2]
    # column 1 uses w[1,0] as the outer factor (faithful to source)
    c1[1] = w[1, 0] * w[1, 0]
    c2[1] = w[1, 1] * w[1, 0]

    j = np.arange(D)
    G = np.zeros((D, D), np.float64)
    G[j, (j + 1) % D] += c1
    G[j, (j - 2) % D] -= c2

    # Row-rotate everything by -1 so partition p of the device result holds
    # output feature (p+1) mod D; the host un-rotates on assembly.
    G_rot = np.roll(G, -1, axis=0)
    W_rot = np.roll(np.asarray(W_lin, np.float64), -1, axis=0)
    b_rot = np.roll(np.asarray(b_lin, np.float32), -1)
    gt = np.ascontiguousarray(G_rot.T).astype(np.float16)  # lhsT for g_rot
    wt = np.ascontiguousarray(W_rot.T).astype(np.float16)  # lhsT for mm_rot
    b = b_rot.reshape(D, 1)
    return {"gt": gt, "wt": wt, "b": b}


_PROGRAM_CACHE: dict[int, object] = {}
TRACE = False      # test-only: capture NTFF profile on the next kernel() call
TRACE_DIR = None   # test-only: where to keep NTFF/perfetto artifacts
LAST_RESULT = None  # test-only: BassKernelResults of the last run


def _get_program(ncols: int):
    if ncols not in _PROGRAM_CACHE:
        _PROGRAM_CACHE[ncols] = build_program(ncols)
    return _PROGRAM_CACHE[ncols]


def kernel(**inputs) -> np.ndarray:
    inp = np.asarray(inputs["inp"])
    w = np.asarray(inputs["w"], np.float32)
    W_lin = np.asarray(inputs["W_lin"], np.float32)
    b_lin = np.asarray(inputs["b_lin"], np.float32)

    B = inp.shape[0]
    assert inp.shape[1] == D and B % N_CORES == 0
    ncols = B // N_CORES  # original rows per core = device free-dim columns

    consts = make_consts(w, W_lin, b_lin)
    inp16 = inp.astype(np.float16)
    shards = inp16.reshape(N_CORES, ncols, D)

    nc = _get_program(ncols)
    in_maps = [
        {"at": np.ascontiguousarray(shards[i].T), **consts} for i in range(N_CORES)
    ]
    res = run_bass_kernel_spmd(
        nc, in_maps, list(range(N_CORES)), trace=TRACE, tmpdir=TRACE_DIR
    )
    global LAST_RESULT
    LAST_RESULT = res

    out = np.empty((B, D), np.float32)
    for i in range(N_CORES):
        # un-rotate: device partition p holds output feature (p+1) mod D
        out[i * ncols : (i + 1) * ncols] = np.roll(res.results[i]["out"], 1, axis=0).T
    return out


if __name__ == "__main__":
    # quick smoke test on random data vs numpy
    rng = np.random.default_rng(0)
    B = N_CORES * TCOLS * 2
    inp = rng.standard_normal((B, D)).astype(np.float32)
    w = rng.random((D, 3)).astype(np.float32)
    W_lin = (rng.standard_normal((D, D)) / np.sqrt(D)).astype(np.float32)
    b_lin = (rng.standard_normal(D) * 0.01).astype(np.float32)
    dt = np.ones(1, np.float32)

    actual = kernel(inp=inp, dt=dt, w=w, W_lin=W_lin, b_lin=b_lin)

    a = inp.astype(np.float64)
    c1 = (w[:, 0] * w[:, 2]).astype(np.float64)
    c2 = (w[:, 1] * w[:, 2]).astype(np.float64)
    c1[1] = float(w[1, 0]) * float(w[1, 0])
    c2[1] = float(w[1, 1]) * float(w[1, 0])
    ap1 = np.roll(a, -1, 1)
    am2 = np.roll(a, 2, 1)
    am1 = np.roll(a, 1, 1)
    x = (c1 * ap1 - c2 * am2) * am1
    expected = x + a @ W_lin.astype(np.float64).T + b_lin
    err = np.abs(actual - expected).max() / np.abs(expected).max()
    print("scale-relative absmax err:", err)


# revision 14
# speedup vs baseline: 1.8553x; 1.6549x over previous
"""Trainium2 Bass kernel for the BINN convnet problem (fp16, transposed layout).

Computation (per row b of inp, all column indices mod D=128):
    g[b, j]  = c1[j] * a[b, j+1] - c2[j] * a[b, j-2]
    x[b, j]  = g[b, j] * a[b, j-1]
    out      = x + a @ W_lin.T + b_lin
with c1[j] = w[j,0]*w[j,2], c2[j] = w[j,1]*w[j,2], except j==1 where the
outer factor is w[1,0] instead of w[1,2].  g is linear in a: g = a @ G.T for
a constant banded G.

The correctness gate is scale-relative absmax < 2e-2; fp16 end-to-end
measures ~7e-4, so all HBM traffic runs at 2 bytes/elem — half the fp32
baseline (32 MiB/core instead of 64 MiB/core; DMA is the roofline).

Strategy: pure data parallel across 8 NeuronCores (batch split).  The host
pre-transposes each 65536-row shard to A^T [128, 65536] fp16, so on-device:

  1. Loads are plain contiguous DMAs (16 KiB per partition per tile),
     no transposes anywhere on device.
  2. g^T = G @ A^T and mm^T = W_lin @ A^T are matmuls with *constant*
     stationary operands (G^T, W_lin^T) and A^T chunks moving, N=512 per
     PSUM bank, natural transposed output layout.
  3. The stencil roll j-1 is absorbed into the constants: the device
     computes the row-rotated output out_dev[p] = out[(p+1) mod 128] using
     rolled G, W_lin, b_lin, so the DVE multiply x_dev = g_rot * A^T is
     perfectly partition-aligned (PSUM partition offsets are illegal) and
     needs no wrap op.  The host un-rotates for free on assembly.
  4. The W matmul accumulates mm on top of x in PSUM (start=False).
     PSUM "zero pending" bits make a bank's first matmul write after
     start=True overwrite instead of accumulate; since the x banks never
     see start=True, their initial pending state is whatever the previous
     NEFF left, which corrupted each bank's first chunk.  A dummy
     full-region start=True matmul per x bank at init clears the bits.
  5. ScalarE evacuates PSUM -> SBUF fp16 adding b_lin, which in transposed
     layout is a per-partition activation bias.
  6. fp16 stores; the host transposes back and upcasts to fp32.  Stores are
     emitted on the ACT queue two 2-chunk groups late so their semaphore
     wait never stalls ACT's queue ahead of pending evacuations.

PE sees [G,G,W,W] per 2-chunk group - one stationary reload per operand
per group; GpSimd stays idle (its fp16 SBUF add measured only ~58 G
elem/s, which made it the bottleneck when it held the final add).
"""

import os
import sys

import numpy as np

if os.path.isdir("/opt/trn_rl_repo") and "/opt/trn_rl_repo" not in sys.path:
    sys.path.insert(0, "/opt/trn_rl_repo")

import concourse.mybir as mybir
import concourse.tile as tile
from concourse import bacc
from concourse.bass_utils import run_bass_kernel_spmd

D = 128          # feature dim
N_CORES = 8
CHUNK = 512      # columns (= batch rows) per PSUM bank / matmul
TCOLS = 8192     # columns per DMA tile (2 MiB fp16)
F16 = mybir.dt.float16
F32 = mybir.dt.float32


def build_program(ncols: int):
    """Build the single-core Bass program (SPMD across cores).

    ncols = rows of the original problem handled by this core; the device
    works on A^T [128, ncols] fp16.
    """
    assert ncols % TCOLS == 0
    ntiles = ncols // TCOLS
    cpt = TCOLS // CHUNK          # chunks per tile (16)
    nchunks = ntiles * cpt
    GRP = 2                       # chunks per PE stationary group

    nc = bacc.Bacc("TRN2", debug=False, target_bir_lowering=False)

    at_d = nc.declare_dram_parameter("at", [D, ncols], F16, isOutput=False)
    gt_d = nc.declare_dram_parameter("gt", [D, D], F16, isOutput=False)
    wt_d = nc.declare_dram_parameter("wt", [D, D], F16, isOutput=False)
    b_d = nc.declare_dram_parameter("b", [D, 1], F32, isOutput=False)
    out_d = nc.declare_dram_parameter("out", [D, ncols], F16, isOutput=True)

    with tile.TileContext(nc) as tc:
        with (
            tc.tile_pool(name="const", bufs=1) as const_pool,
            tc.tile_pool(name="a_sb", bufs=3) as a_pool,
            tc.tile_pool(name="o_sb", bufs=3) as o_pool,
            tc.tile_pool(name="g_ps", bufs=4, space="PSUM") as g_pool,
            tc.tile_pool(name="x_ps", bufs=4, space="PSUM") as x_pool,
        ):
            gt_sb = const_pool.tile([D, D], F16)
            wt_sb = const_pool.tile([D, D], F16)
            b_sb = const_pool.tile([D, 1], F32)
            dum_sb = const_pool.tile([1, CHUNK], F16)
            nc.sync.dma_start(out=gt_sb[:], in_=gt_d[:, :])
            nc.sync.dma_start(out=wt_sb[:], in_=wt_d[:, :])
            nc.sync.dma_start(out=b_sb[:], in_=b_d[:, :])
            nc.vector.memset(dum_sb[:], 0.0)

            # Clear the x banks' PSUM zero-pending bits: one full-region
            # start=True matmul per bank (values are overwritten later).
            warm = []
            for _ in range(4):
                x_ps = x_pool.tile([D, CHUNK], F32, tag="x")
                nc.tensor.matmul(
                    out=x_ps[:],
                    lhsT=gt_sb[0:1, :],
                    rhs=dum_sb[:],
                    start=True,
                    stop=True,
                )
                warm.append(x_ps)
            del warm

            tiles = {}  # tile t -> (at_sb, o_sb)
            st = {}     # chunk k -> (at_sb, o_sb, col, x_ps)
            pending_stores = []  # [(emit_after_k0, t, o_sb), ...]

            def tile_of(k):
                t, c = divmod(k, cpt)
                if c == 0:
                    at_sb = a_pool.tile([D, TCOLS], F16, tag="at")
                    nc.sync.dma_start(
                        out=at_sb[:], in_=at_d[:, t * TCOLS : (t + 1) * TCOLS]
                    )
                    o_sb = o_pool.tile([D, TCOLS], F16, tag="o")
                    tiles[t] = (at_sb, o_sb)
                return tiles[t]

            def emit_front(k):
                """G-matmul + DVE stencil multiply for chunk k."""
                at_sb, o_sb = tile_of(k)
                col = (k % cpt) * CHUNK
                g_ps = g_pool.tile([D, CHUNK], F32, tag="g")
                nc.tensor.matmul(
                    out=g_ps[:],
                    lhsT=gt_sb[:],
                    rhs=at_sb[:, col : col + CHUNK],
                    start=True,
                    stop=True,
                )
                # x_dev[p] = g[p+1]*a[p]: rotation baked into G_rot, so this
                # is a single partition-aligned multiply.
                x_ps = x_pool.tile([D, CHUNK], F32, tag="x")
                nc.vector.tensor_mul(
                    out=x_ps[:], in0=g_ps[:], in1=at_sb[:, col : col + CHUNK]
                )
                st[k] = (at_sb, o_sb, col, x_ps)

            def emit_back(k):
                """W-matmul accumulate + bias evac for chunk k."""
                at_sb, o_sb, col, x_ps = st.pop(k)
                nc.tensor.matmul(
                    out=x_ps[:],
                    lhsT=wt_sb[:],
                    rhs=at_sb[:, col : col + CHUNK],
                    start=False,
                    stop=True,
                    skip_group_check=True,
                )
                # out = x + mm + b_lin (per-partition bias), PSUM -> SBUF fp16
                nc.scalar.add(
                    out=o_sb[:, col : col + CHUNK], in_=x_ps[:], add=b_sb[:, 0:1]
                )
                t, c = divmod(k, cpt)
                if c == cpt - 1:
                    pending_stores.append((k + 2 * GRP, t, o_sb))

            # 2-chunk groups, software-pipelined by one group: PE stream is
            # [G(k),G(k+1),W(k-2),W(k-1)] so the PE never waits on the DVE
            # round-trip and stationary reloads amortize over the group.
            for k0 in range(0, nchunks + GRP, GRP):
                for k in range(k0, k0 + GRP):
                    if k < nchunks:
                        emit_front(k)
                for k in range(k0 - GRP, k0):
                    if 0 <= k < nchunks:
                        emit_back(k)
                while pending_stores and pending_stores[0][0] <= k0:
                    _, t, o_sb = pending_stores.pop(0)
                    nc.scalar.dma_start(
                        out=out_d[:, t * TCOLS : (t + 1) * TCOLS], in_=o_sb[:]
                    )
            for _, t, o_sb in pending_stores:
                nc.scalar.dma_start(
                    out=out_d[:, t * TCOLS : (t + 1) * TCOLS], in_=o_sb[:]
                )

    nc.compile()
    return nc


def make_consts(w: np.ndarray, W_lin: np.ndarray, b_lin: np.ndarray):
    """Host-side constant preparation (all tiny)."""
    w = np.asarray(w, np.float64)
    c1 = w[:, 0] * w[:, 2]
    c2 = w[:, 1] * w[:, 2]
    # column 1 uses w[1,0] as the outer factor (faithful to source)
    c1[1] = w[1, 0] * w[1, 0]
    c2[1] = w[1, 1] * w[1, 0]

    j = np.arange(D)
    G = np.zeros((D, D), np.float64)
    G[j, (j + 1) % D] += c1
    G[j, (j - 2) % D] -= c2

    # Row-rotate everything by -1 so partition p of the device result holds
    # output feature (p+1) mod D; the host un-rotates on assembly.
    G_rot = np.roll(G, -1, axis=0)
    W_rot = np.roll(np.asarray(W_lin, np.float64), -1, axis=0)
    b_rot = np.roll(np.asarray(b_lin, np.float32), -1)
    gt = np.ascontiguousarray(G_rot.T).astype(np.float16)  # lhsT for g_rot
    wt = np.ascontiguousarray(W_rot.T).astype(np.float16)  # lhsT for mm_rot
    b = b_rot.reshape(D, 1)
    return {"gt": gt, "wt": wt, "b": b}


_PROGRAM_CACHE: dict[int, object] = {}
TRACE = False      # test-only: capture NTFF profile on the next kernel() call
TRACE_DIR = None   # test-only: where to keep NTFF/perfetto artifacts
LAST_RESULT = None  # test-only: BassKernelResults of the last run


def _get_program(ncols: int):
    if ncols not in _PROGRAM_CACHE:
        _PROGRAM_CACHE[ncols] = build_program(ncols)
    return _PROGRAM_CACHE[ncols]


def kernel(**inputs) -> np.ndarray:
    inp = np.asarray(inputs["inp"])
    w = np.asarray(inputs["w"], np.float32)
    W_lin = np.asarray(inputs["W_lin"], np.float32)
    b_lin = np.asarray(inputs["b_lin"], np.float32)

    B = inp.shape[0]
    assert inp.shape[1] == D and B % N_CORES == 0
    ncols = B // N_CORES  # original rows per core = device free-dim columns

    consts = make_consts(w, W_lin, b_lin)
    inp16 = inp.astype(np.float16)
    shards = inp16.reshape(N_CORES, ncols, D)

    nc = _get_program(ncols)
    in_maps = [
        {"at": np.ascontiguousarray(shards[i].T), **consts} for i in range(N_CORES)
    ]
    res = run_bass_kernel_spmd(
        nc, in_maps, list(range(N_CORES)), trace=TRACE, tmpdir=TRACE_DIR
    )
    global LAST_RESULT
    LAST_RESULT = res

    out = np.empty((B, D), np.float32)
    for i in range(N_CORES):
        # un-rotate: device partition p holds output feature (p+1) mod D
        out[i * ncols : (i + 1) * ncols] = np.roll(res.results[i]["out"], 1, axis=0).T
    return out


if __name__ == "__main__":
    # quick smoke test on random data vs numpy
    rng = np.random.default_rng(0)
    B = N_CORES * TCOLS * 2
    inp = rng.standard_normal((B, D)).astype(np.float32)
    w = rng.random((D, 3)).astype(np.float32)
    W_lin = (rng.standard_normal((D, D)) / np.sqrt(D)).astype(np.float32)
    b_lin = (rng.standard_normal(D) * 0.01).astype(np.float32)
    dt = np.ones(1, np.float32)

    actual = kernel(inp=inp, dt=dt, w=w, W_lin=W_lin, b_lin=b_lin)

    a = inp.astype(np.float64)
    c1 = (w[:, 0] * w[:, 2]).astype(np.float64)
    c2 = (w[:, 1] * w[:, 2]).astype(np.float64)
    c1[1] = float(w[1, 0]) * float(w[1, 0])
    c2[1] = float(w[1, 1]) * float(w[1, 0])
    ap1 = np.roll(a, -1, 1)
    am2 = np.roll(a, 2, 1)
    am1 = np.roll(a, 1, 1)
    x = (c1 * ap1 - c2 * am2) * am1
    expected = x + a @ W_lin.astype(np.float64).T + b_lin
    err = np.abs(actual - expected).max() / np.abs(expected).max()
    print("scale-relative absmax err:", err)
